# revision 17
# baseline (speedup 1.0000x reference)
"""NodeWiseMamba Trainium2 kernel (Bass/Tile), 8-core SPMD data parallel.

Sharding: N=4096 nodes split evenly across 8 NeuronCores (512 nodes/core).
Every node's T=24 sequence is independent; all weights replicated per core.

Layout convention on-chip: activations are [feature(partition), rows(free)]
where rows = packed (node, t). All dense projections run on the PE with the
weight stationary. The selective scan runs on the DVE via tensor_tensor_scan
(h = dA*h + dBx along the packed free axis; dA is zeroed at each node's t=0
column so state resets across node boundaries inside one scan op).
"""

import sys

if "/opt/trn_rl_repo" not in sys.path:
    sys.path.insert(0, "/opt/trn_rl_repo")

from contextlib import ExitStack

import numpy as np

import concourse.bass as bass
import concourse.tile as tile
from concourse import bacc
from concourse import mybir
from concourse.masks import make_identity

F32 = mybir.dt.float32
AF = mybir.ActivationFunctionType
ALU = mybir.AluOpType
AX = mybir.AxisListType

N_TOTAL, T, IND = 4096, 24, 256
HID, DIN, DST, DTR = 256, 256, 16, 16
NCORES = 8
NPC = N_TOTAL // NCORES  # nodes per core
NODE_CHUNK = 32          # nodes per processing chunk


def build_program(npc=NPC, node_chunk=NODE_CHUNK, debug=False):
    RC = node_chunk * T          # rows per chunk (768)
    NCH = npc // node_chunk      # chunks
    NSL = 2                      # matmul free-dim slices per chunk
    SL = RC // NSL               # 384 (fits one PSUM bank in f32)
    MMW = max(SL, npc)           # psum matmul tile width

    nc = bacc.Bacc(None)
    x_d = nc.declare_dram_parameter("x_hist", [npc, T, IND], F32, isOutput=False)
    Wi_d = nc.declare_dram_parameter("Wi", [IND, HID], F32, isOutput=False)
    bi_d = nc.declare_dram_parameter("bi", [HID], F32, isOutput=False)
    Win_d = nc.declare_dram_parameter("Win", [HID, 2 * DIN], F32, isOutput=False)
    convW_d = nc.declare_dram_parameter("convW", [DIN, 1, 4], F32, isOutput=False)
    convb_d = nc.declare_dram_parameter("convb", [DIN], F32, isOutput=False)
    Wx_d = nc.declare_dram_parameter("Wx", [DIN, DTR + 2 * DST], F32, isOutput=False)
    Wdt_d = nc.declare_dram_parameter("Wdt", [DTR, DIN], F32, isOutput=False)
    bdt_d = nc.declare_dram_parameter("bdt", [DIN], F32, isOutput=False)
    Alog_d = nc.declare_dram_parameter("A_log", [DIN, DST], F32, isOutput=False)
    Dp_d = nc.declare_dram_parameter("Dp", [DIN], F32, isOutput=False)
    Wso_d = nc.declare_dram_parameter("Wssm_out", [DIN, HID], F32, isOutput=False)
    Wat_d = nc.declare_dram_parameter("Wattn", [HID, 1], F32, isOutput=False)
    Wo_d = nc.declare_dram_parameter("Wo", [HID, HID], F32, isOutput=False)
    bo_d = nc.declare_dram_parameter("bo", [HID], F32, isOutput=False)
    out_d = nc.declare_dram_parameter("out", [npc, HID], F32, isOutput=True)
    dbg = {}
    if debug:
        RC0 = node_chunk * T
        for nm, shp in [("d_xT", [2, 128, RC0]), ("d_x1", [2, 128, RC0]),
                        ("d_xm", [2, 128, RC0]), ("d_dt", [2, 128, RC0]),
                        ("d_BC", [2, DTR, RC0]), ("d_y", [2, 128, RC0]),
                        ("d_u2", [2, 128, RC0]), ("d_lg", [1, RC0]),
                        ("d_Z", [2, 128, node_chunk])]:
            dbg[nm] = nc.declare_dram_parameter(nm, shp, F32, isOutput=True)

    xr = x_d[:].rearrange("n t f -> (n t) f")

    with tile.TileContext(nc) as tc:
        with ExitStack() as ctx:
            wp = ctx.enter_context(tc.tile_pool(name="weights", bufs=1))
            pin = ctx.enter_context(tc.tile_pool(name="pin", bufs=16))
            pbig = ctx.enter_context(tc.tile_pool(name="pbig", bufs=2))
            pscan = ctx.enter_context(tc.tile_pool(name="pscan", bufs=2))
            psm = ctx.enter_context(tc.tile_pool(name="psm", bufs=2))
            ps_tr = ctx.enter_context(tc.tile_pool(name="ps_tr", bufs=2, space="PSUM"))
            ps_mm = ctx.enter_context(tc.tile_pool(name="ps_mm", bufs=2, space="PSUM"))
            ps_rep = ctx.enter_context(tc.tile_pool(name="ps_rep", bufs=4, space="PSUM"))

            # ---- weights / constants (loaded once) ----
            _wc = [0]

            def load2(dram_ap, cols):
                _wc[0] += 1
                ts_ = [wp.tile([128, cols], F32, name=f"w2_{_wc[0]}_{k2}") for k2 in range(2)]
                for k in range(2):
                    nc.sync.dma_start(out=ts_[k][:], in_=dram_ap[k * 128:(k + 1) * 128, :])
                return ts_

            Wi_sb = load2(Wi_d[:], HID)
            Win_sb = load2(Win_d[:], 2 * DIN)
            Wx_sb = load2(Wx_d[:], DTR + 2 * DST)
            Wso_sb = load2(Wso_d[:], HID)
            Wat_sb = load2(Wat_d[:], 1)
            Wo_sb = load2(Wo_d[:], HID)
            Wdt_sb = wp.tile([DTR, DIN], F32)
            nc.sync.dma_start(out=Wdt_sb[:], in_=Wdt_d[:])

            def loadvec(dram_ap):
                _wc[0] += 1
                t = wp.tile([128, 2], F32, name=f"vec_{_wc[0]}")
                nc.sync.dma_start(out=t[:], in_=dram_ap.rearrange("(c p) -> p c", p=128))
                return t

            bi_sb = loadvec(bi_d[:])
            convb_sb = loadvec(convb_d[:])
            bdt_sb = loadvec(bdt_d[:])
            Dp_sb = loadvec(Dp_d[:])
            bo_sb = loadvec(bo_d[:])

            convW_sb = wp.tile([128, 2, 4], F32)
            nc.sync.dma_start(
                out=convW_sb[:], in_=convW_d[:].rearrange("(c p) one k -> p c (one k)", p=128)
            )
            Alog_sb = wp.tile([128, 2, DST], F32)
            nc.sync.dma_start(
                out=Alog_sb[:], in_=Alog_d[:].rearrange("(c p) s -> p c s", p=128)
            )
            negA_sb = wp.tile([128, 2, DST], F32)
            nc.scalar.activation(negA_sb[:], Alog_sb[:], AF.Exp)
            nc.vector.tensor_scalar_mul(negA_sb[:], negA_sb[:], -1.0)

            ones_sb = wp.tile([1, 128], F32)
            nc.gpsimd.memset(ones_sb[:], 1.0)
            # selector for broadcasting row s of a [16, X] tile to 128 partitions:
            # sel[k, s*128+d] = 1 if k == s else 0;  rep = sel_s.T @ B  (K=16)
            sel_sb = wp.tile([DST, DST * 128], F32)
            nc.gpsimd.memset(sel_sb[:], 0.0)
            # sel[p, s, d] = 1 iff p == s:  expr = p - s, fill where expr == 0
            nc.gpsimd.affine_select(
                out=sel_sb[:].rearrange("p (s d) -> p s d", d=128),
                in_=sel_sb[:].rearrange("p (s d) -> p s d", d=128),
                compare_op=ALU.not_equal,
                fill=1.0,
                base=0,
                pattern=[[-1, DST], [0, 128]],
                channel_multiplier=1,
            )
            ident = wp.tile([128, 128], F32)
            make_identity(nc, ident[:])

            Z_sb = [wp.tile([128, npc], F32, name=f"Z_{m2}") for m2 in range(2)]

            # "Probe" matmuls: make the PE observe a tile's producer semaphore
            # via a 1-col matmul, so later real matmuls carry <=1 sync wait
            # (the transpose LDW slot only fits one wait condition).
            def pe_probe(ap_2d):
                # standalone bf16-bitcast LDWEIGHTS: reads the tile (so the PE
                # observes its producer's semaphore) but writes nothing.
                nc.tensor.ldweights(ap_2d[:, 0:1].bitcast(mybir.dt.bfloat16))

            for _w in (Wi_sb + Win_sb + Wx_sb + Wso_sb + Wo_sb + [Wdt_sb]):
                pe_probe(_w[:, 0:1] if _w.shape[1] >= 1 else _w)
            pe_probe(Wat_sb[0][:])
            pe_probe(Wat_sb[1][:])
            # observe the gpsimd (Pool) ticks for sel/ident/ones
            pe_probe(ident[:])

            # ---- per-chunk pipeline ----
            for c in range(NCH):
                r0 = c * RC
                # load + transpose x -> xT [feat, rows]
                xT = [pbig.tile([128, RC], F32, tag=f"xT{k}", name=f"xT{k}", bufs=1) for k in range(2)]
                for i in range(RC // 128):
                    xin = pin.tile([128, IND], F32, tag="xin", name="xin")
                    nc.sync.dma_start(out=xin[:], in_=xr[r0 + i * 128: r0 + (i + 1) * 128, :])
                    pe_probe(xin[:, 0:1])
                    for k in range(2):
                        pt = ps_tr.tile([128, 128], F32, tag="tr", name="tr")
                        nc.tensor.transpose(pt[:], xin[:, k * 128:(k + 1) * 128], ident[:])
                        nc.scalar.copy(xT[k][:, i * 128:(i + 1) * 128], pt[:])

                if debug and c == 0:
                    for k in range(2):
                        nc.sync.dma_start(out=dbg["d_xT"][k], in_=xT[k][:])
                # MM1: x1 = Wi.T @ xT + bi
                x1 = [pbig.tile([128, RC], F32, tag=f"x1_{m}", name=f"x1_{m}", bufs=1) for m in range(2)]
                for m in range(2):
                    for j in range(NSL):
                        pm = ps_mm.tile([128, MMW], F32, tag="mm", name="mm")
                        for k in range(2):
                            nc.tensor.matmul(
                                pm[:, :SL], Wi_sb[k][:, m * 128:(m + 1) * 128],
                                xT[k][:, j * SL:(j + 1) * SL],
                                start=(k == 0), stop=(k == 1),
                            )
                        nc.scalar.activation(
                            x1[m][:, j * SL:(j + 1) * SL], pm[:, :SL], AF.Identity,
                            bias=bi_sb[:, m:m + 1],
                        )

                if debug and c == 0:
                    for k in range(2):
                        nc.sync.dma_start(out=dbg["d_x1"][k], in_=x1[k][:])
                # MM2: [xm_pre | zgate] = Win.T @ x1
                xmp = [pbig.tile([128, RC], F32, tag=f"xmp{d}", name=f"xmp{d}") for d in range(2)]
                zg = [pbig.tile([128, RC], F32, tag=f"zg{d}", name=f"zg{d}") for d in range(2)]
                dest = [xmp[0], xmp[1], zg[0], zg[1]]
                for m in range(4):
                    for j in range(NSL):
                        pm = ps_mm.tile([128, MMW], F32, tag="mm", name="mm")
                        for k in range(2):
                            nc.tensor.matmul(
                                pm[:, :SL], Win_sb[k][:, m * 128:(m + 1) * 128],
                                x1[k][:, j * SL:(j + 1) * SL],
                                start=(k == 0), stop=(k == 1),
                            )
                        nc.scalar.copy(dest[m][:, j * SL:(j + 1) * SL], pm[:, :SL])

                # causal depthwise conv (4 taps) + silu -> xm
                xm = [pbig.tile([128, RC], F32, tag=f"xm{d}", name=f"xm{d}") for d in range(2)]
                for d in range(2):
                    acc = pbig.tile([128, RC], F32, tag="convacc", name="convacc", bufs=1)
                    a3 = acc[:].rearrange("p (n t) -> p n t", t=T)
                    s3 = xmp[d][:].rearrange("p (n t) -> p n t", t=T)
                    nc.vector.tensor_scalar_mul(acc[:], xmp[d][:], convW_sb[:, d:d + 1, 3:4])
                    for sh, w_idx in ((1, 2), (2, 1), (3, 0)):
                        nc.vector.scalar_tensor_tensor(
                            out=a3[:, :, sh:], in0=s3[:, :, :T - sh],
                            scalar=convW_sb[:, d:d + 1, w_idx:w_idx + 1],
                            in1=a3[:, :, sh:], op0=ALU.mult, op1=ALU.add,
                        )
                    nc.scalar.activation(xm[d][:], acc[:], AF.Silu, bias=convb_sb[:, d:d + 1])

                if debug and c == 0:
                    for k in range(2):
                        nc.sync.dma_start(out=dbg["d_xm"][k], in_=xm[k][:])
                # MM3: dt_raw, B, C = Wx.T @ xm (three M=16 chains, each at partition 0)
                dtrBC = []
                for g in range(3):
                    sb = psm.tile([DTR, RC], F32, tag=f"dtrbc{g}", name=f"dtrbc{g}")
                    for j in range(NSL):
                        pm = ps_mm.tile([128, MMW], F32, tag="mm", name="mm")
                        for k in range(2):
                            nc.tensor.matmul(
                                pm[:DTR, :SL], Wx_sb[k][:, g * 16:(g + 1) * 16],
                                xm[k][:, j * SL:(j + 1) * SL],
                                start=(k == 0), stop=(k == 1),
                            )
                        nc.scalar.copy(sb[:, j * SL:(j + 1) * SL], pm[:DTR, :SL])
                    dtrBC.append(sb)
                dtraw_sb, B_sb, C_sb = dtrBC

                # MM4: dt = softplus(Wdt.T @ dt_raw + bdt); dtx = dt * xm
                dt_sb = [pbig.tile([128, RC], F32, tag=f"dt{d}", name=f"dt{d}") for d in range(2)]
                dtm_sb = [pbig.tile([128, RC], F32, tag=f"dtm{d}", name=f"dtm{d}") for d in range(2)]
                dtx_sb = [pbig.tile([128, RC], F32, tag=f"dtx{d}", name=f"dtx{d}") for d in range(2)]
                for d in range(2):
                    for j in range(NSL):
                        pm = ps_mm.tile([128, MMW], F32, tag="mm", name="mm")
                        nc.tensor.matmul(
                            pm[:, :SL], Wdt_sb[:, d * 128:(d + 1) * 128],
                            dtraw_sb[:, j * SL:(j + 1) * SL], start=True, stop=True,
                        )
                        # softplus(x) = ln(1 + e^x); ACT has no softplus table set
                        nc.scalar.activation(
                            dt_sb[d][:, j * SL:(j + 1) * SL], pm[:, :SL], AF.Exp,
                            bias=bdt_sb[:, d:d + 1],
                        )
                        nc.scalar.activation(
                            dt_sb[d][:, j * SL:(j + 1) * SL],
                            dt_sb[d][:, j * SL:(j + 1) * SL], AF.Ln, bias=1.0,
                        )
                    nc.vector.tensor_mul(dtx_sb[d][:], dt_sb[d][:], xm[d][:])
                    # masked copy of dt: huge value at each node's t=0 column so
                    # exp(negA * dtm) == 0 there -> scan state resets per node
                    nc.vector.tensor_copy(dtm_sb[d][:], dt_sb[d][:])
                    nc.gpsimd.memset(
                        dtm_sb[d][:].rearrange("p (n t) -> p n t", t=T)[:, :, 0:1], 1.0e4
                    )

                if debug and c == 0:
                    for d2 in range(2):
                        nc.sync.dma_start(out=dbg["d_dt"][d2], in_=dt_sb[d2][:])
                    nc.sync.dma_start(out=dbg["d_BC"][0], in_=B_sb[:])
                    nc.sync.dma_start(out=dbg["d_BC"][1], in_=C_sb[:])
                # selective scan over s-states
                y_sb = [pbig.tile([128, RC], F32, tag=f"y{d}", name=f"y{d}") for d in range(2)]
                for s in range(DST):
                    Brep, Crep = [], []
                    for j in range(NSL):
                        for src, acc_list in ((B_sb, Brep), (C_sb, Crep)):
                            rp = ps_rep.tile([128, SL], F32, tag="rep", name="rep")
                            nc.tensor.matmul(
                                rp[:], sel_sb[:, s * 128:(s + 1) * 128],
                                src[:, j * SL:(j + 1) * SL],
                                start=True, stop=True,
                            )
                            acc_list.append(rp)
                    for d in range(2):
                        dA = pscan.tile([128, RC], F32, tag="dA", name="dA")
                        nc.scalar.activation(
                            dA[:], dtm_sb[d][:], AF.Exp, scale=negA_sb[:, d:d + 1, s:s + 1]
                        )
                        dBx = pscan.tile([128, RC], F32, tag="dBx", name="dBx")
                        for j in range(NSL):
                            nc.vector.tensor_mul(
                                dBx[:, j * SL:(j + 1) * SL],
                                dtx_sb[d][:, j * SL:(j + 1) * SL], Brep[j][:],
                            )
                        h = pscan.tile([128, RC], F32, tag="h", name="h")
                        nc.vector.tensor_tensor_scan(
                            h[:], dA[:], dBx[:], 0.0, ALU.mult, ALU.add
                        )
                        if s == 0:
                            for j in range(NSL):
                                nc.vector.tensor_mul(
                                    y_sb[d][:, j * SL:(j + 1) * SL],
                                    h[:, j * SL:(j + 1) * SL], Crep[j][:],
                                )
                        else:
                            yp = pscan.tile([128, RC], F32, tag="yp", name="yp")
                            for j in range(NSL):
                                nc.vector.tensor_mul(
                                    yp[:, j * SL:(j + 1) * SL],
                                    h[:, j * SL:(j + 1) * SL], Crep[j][:],
                                )
                            nc.vector.tensor_add(y_sb[d][:], y_sb[d][:], yp[:])

                if debug and c == 0:
                    for d2 in range(2):
                        nc.sync.dma_start(out=dbg["d_y"][d2], in_=y_sb[d2][:])
                # gate (in place): y = (y + xm*Dp) * silu(zgate)
                y2 = y_sb
                for d in range(2):
                    nc.vector.scalar_tensor_tensor(
                        out=y_sb[d][:], in0=xm[d][:], scalar=Dp_sb[:, d:d + 1],
                        in1=y_sb[d][:], op0=ALU.mult, op1=ALU.add,
                    )
                    nc.scalar.activation(zg[d][:], zg[d][:], AF.Silu)
                    nc.vector.tensor_mul(y_sb[d][:], y_sb[d][:], zg[d][:])

                # MM5: u2 = Wssm_out.T @ y2
                u2 = [pbig.tile([128, RC], F32, tag=f"u2_{m}", name=f"u2_{m}", bufs=1) for m in range(2)]
                for m in range(2):
                    for j in range(NSL):
                        pm = ps_mm.tile([128, MMW], F32, tag="mm", name="mm")
                        for k in range(2):
                            nc.tensor.matmul(
                                pm[:, :SL], Wso_sb[k][:, m * 128:(m + 1) * 128],
                                y2[k][:, j * SL:(j + 1) * SL],
                                start=(k == 0), stop=(k == 1),
                            )
                        nc.scalar.copy(u2[m][:, j * SL:(j + 1) * SL], pm[:, :SL])

                if debug and c == 0:
                    for m2 in range(2):
                        nc.sync.dma_start(out=dbg["d_u2"][m2], in_=u2[m2][:])
                # attention logits + division-deferred softmax pooling
                lg = psm.tile([1, RC], F32, tag="lg", name="lg", bufs=1)
                for j in range(NSL):
                    pm = ps_mm.tile([128, MMW], F32, tag="mm", name="mm")
                    for k in range(2):
                        nc.tensor.matmul(
                            pm[0:1, :SL], Wat_sb[k][:], u2[k][:, j * SL:(j + 1) * SL],
                            start=(k == 0), stop=(k == 1),
                        )
                    nc.scalar.copy(lg[:, j * SL:(j + 1) * SL], pm[0:1, :SL])
                e_sb = psm.tile([1, RC], F32, tag="e", name="e", bufs=1)
                nc.scalar.activation(e_sb[:], lg[:], AF.Exp)
                e3 = e_sb[:].rearrange("p (n t) -> p n t", t=T)
                S_t = psm.tile([1, node_chunk], F32, tag="S", name="S")
                nc.vector.tensor_reduce(S_t[:], e3, axis=AX.X, op=ALU.add)
                rS = psm.tile([1, node_chunk], F32, tag="rS", name="rS")
                nc.vector.reciprocal(rS[:], S_t[:])
                wrep = []
                for j in range(NSL):
                    rp = ps_rep.tile([128, SL], F32, tag="rep", name="rep")
                    nc.tensor.matmul(
                        rp[:], ones_sb[:], e_sb[0:1, j * SL:(j + 1) * SL],
                        start=True, stop=True,
                    )
                    wrep.append(rp)
                rrep = ps_rep.tile([128, SL], F32, tag="rep", name="rep")
                nc.tensor.matmul(rrep[:, :node_chunk], ones_sb[:], rS[:], start=True, stop=True)
                rrep_sb = psm.tile([128, node_chunk], F32, tag="rrep_sb", name="rrep_sb")
                nc.scalar.copy(rrep_sb[:], rrep[:, :node_chunk])
                for m in range(2):
                    for j in range(NSL):
                        nc.vector.tensor_mul(
                            u2[m][:, j * SL:(j + 1) * SL], u2[m][:, j * SL:(j + 1) * SL], wrep[j][:]
                        )
                    zc = psm.tile([128, node_chunk], F32, tag="zc", name="zc")
                    nc.vector.tensor_reduce(
                        zc[:], u2[m][:].rearrange("p (n t) -> p n t", t=T), axis=AX.X, op=ALU.add
                    )
                    nc.vector.tensor_mul(
                        Z_sb[m][:, c * node_chunk:(c + 1) * node_chunk], zc[:], rrep_sb[:]
                    )

            if debug:
                nc.sync.dma_start(out=dbg["d_lg"][:], in_=lg[:])
                for m2 in range(2):
                    nc.sync.dma_start(out=dbg["d_Z"][m2], in_=Z_sb[m2][:, 0:node_chunk])
            # ---- tail: output projection + exact gelu + transpose out ----
            outT = [wp.tile([128, HID], F32, name=f"outT_{i2}") for i2 in range(npc // 128)]
            for m in range(2):
                pm = ps_mm.tile([128, MMW], F32, tag="mm", name="mm")
                for k in range(2):
                    nc.tensor.matmul(
                        pm[:, :npc], Wo_sb[k][:, m * 128:(m + 1) * 128], Z_sb[k][:],
                        start=(k == 0), stop=(k == 1),
                    )
                zo = psm.tile([128, npc], F32, tag="zo", name="zo")
                nc.scalar.activation(zo[:], pm[:, :npc], AF.Gelu, bias=bo_sb[:, m:m + 1])
                for i in range(npc // 128):
                    pt = ps_tr.tile([128, 128], F32, tag="tr", name="tr")
                    nc.tensor.transpose(pt[:], zo[:, i * 128:(i + 1) * 128], ident[:])
                    nc.scalar.copy(outT[i][:, m * 128:(m + 1) * 128], pt[:])
            for i in range(npc // 128):
                nc.sync.dma_start(out=out_d[i * 128:(i + 1) * 128, :], in_=outT[i][:])

    nc.compile()
    return nc


_CACHE = {}


def _get_program():
    if "nc" not in _CACHE:
        _CACHE["nc"] = build_program()
    return _CACHE["nc"]


def kernel(**inputs):
    from concourse.bass_utils import run_bass_kernel_spmd

    nc = _get_program()
    x = np.asarray(inputs["x_hist"], dtype=np.float32)
    shards = np.split(x, NCORES, axis=0)
    weights = {
        k: np.ascontiguousarray(np.asarray(v, dtype=np.float32))
        for k, v in inputs.items()
        if k not in ("x_hist", "battn")
    }
    in_maps = [dict(weights, x_hist=np.ascontiguousarray(shards[i])) for i in range(NCORES)]
    res = run_bass_kernel_spmd(nc, in_maps, list(range(NCORES)))
    return np.concatenate([res.results[i]["out"] for i in range(NCORES)], axis=0)
